# revision 2
# baseline (speedup 1.0000x reference)
# Trainium2 Bass kernel for nn_MultiHeadAttention (B=2, L=2048, HID=2048, 16 heads).
#
# Sharding: core = (batch, head-group): 2 batches x 4 groups of 4 heads.
# Each core computes q/k/v projections for its 4 heads of its batch, causal
# attention, and a partial output projection (its 512 input channels of
# W_out); the host sums the 4 partial [L, HID] outputs per batch.
#
# Per-core layout choices:
#   - x is shipped pre-transposed (xT = x[b].T, [HID, L], bf16).
#   - Q^T / K^T are computed directly in [D=128, L] layout (head dim on
#     partitions) so score tiles S^T[k,q] come out of the PE ready for the
#     PV matmul with no transposes in the softmax path.
#   - PV: stationary V-chunks [k,128 x d,128] (V^T transposed on the PE),
#     moving E^T tiles [k, q-512] -> out^T [d, q] accumulated in PSUM, which
#     is exactly the stationary layout the output projection needs.
#   - Softmax denominator: a ones-column stationary matmul over the same E^T
#     tiles -> [1, q], reciprocal + partition-broadcast, one fused multiply.
#   - RoPE: head dim of Wq/Wk permuted even-first on the host; rotation is a
#     half-swap + two multiply-adds with [128, L] cos/sin tables. Work is
#     spread over DVE / GpSimd / ACT.
#   - RMSNorm: folded into per-position scales (q-side applied to Q^T before
#     scores, k-side applied as the exp activation's per-partition scale).
#   - Causal mask: strictly-upper S^T tiles skipped; diagonal tiles get a
#     [128,128] additive mask + zero-fill of dead columns.
#   - Output partials stored bf16 (tolerance is 2e-2; bf16 partial sums are
#     ~1e-3), halving store traffic.

import numpy as np
import ml_dtypes

B, L, HID, NH, D = 2, 2048, 2048, 16, 128
NCORES = 8
NB = 2                       # batch shards
NG = 4                       # head-group shards
HPC = NH // NG               # heads per core = 4
NCH = HID // 128             # 16 contraction chunks
NL = 512                     # L tile (free dim)
NLB = L // NL                # 4 L-blocks
ROPE_BASE = 10000.0
EPS = 1e-5
MASK_VAL = -1e9

_BF16 = ml_dtypes.bfloat16
_cache = {}


def _host_constants():
    if "consts" in _cache:
        return _cache["consts"]
    i = np.arange(64, dtype=np.float64)
    inv_freq = ROPE_BASE ** (-2.0 * i / D)                     # [64]
    ang = np.arange(L, dtype=np.float64)[:, None] * inv_freq   # [L, 64]
    cos, sin = np.cos(ang).T, np.sin(ang).T                    # [64, L]
    csa = np.concatenate([cos, cos], axis=0).astype(_BF16)     # [128, L]
    csb = np.concatenate([-sin, sin], axis=0).astype(_BF16)    # [128, L]
    _cache["consts"] = (csa, csb)
    return _cache["consts"]


def _build_nc():
    if "nc" in _cache:
        return _cache["nc"]
    import concourse.bass as bass  # noqa: F401
    from concourse import bacc
    import concourse.tile as tile
    import concourse.mybir as mybir
    from contextlib import ExitStack

    f32 = mybir.dt.float32
    f32r = mybir.dt.float32r
    bf16 = mybir.dt.bfloat16
    EXP = mybir.ActivationFunctionType.Exp
    SQRT = mybir.ActivationFunctionType.Sqrt

    nc = bacc.Bacc("TRN2", target_bir_lowering=False, debug=False,
                   enable_asserts=True)
    xT = nc.dram_tensor("xT", [HID, L], bf16, kind="ExternalInput").ap()
    wqkvT = nc.dram_tensor("wqkvT", [HID, HPC * 3 * D], bf16,
                           kind="ExternalInput").ap()
    woutT = nc.dram_tensor("woutT", [HPC * D, HID], bf16,
                           kind="ExternalInput").ap()
    csa_d = nc.dram_tensor("csa", [D, L], bf16, kind="ExternalInput").ap()
    csb_d = nc.dram_tensor("csb", [D, L], bf16, kind="ExternalInput").ap()
    out_d = nc.dram_tensor("outp", [L, HID], bf16, kind="ExternalOutput").ap()

    with tile.TileContext(nc) as tc, ExitStack() as ctx:
        cpool = ctx.enter_context(tc.tile_pool(name="consts", bufs=1))
        xpool = ctx.enter_context(tc.tile_pool(name="x", bufs=1))
        wpool = ctx.enter_context(tc.tile_pool(name="w", bufs=2))
        qkpool = ctx.enter_context(tc.tile_pool(name="qk", bufs=2))
        vpool = ctx.enter_context(tc.tile_pool(name="v", bufs=2))
        epool = ctx.enter_context(tc.tile_pool(name="e", bufs=17))
        hpool = ctx.enter_context(tc.tile_pool(name="hout", bufs=1))
        spool = ctx.enter_context(tc.tile_pool(name="stage", bufs=2))
        opool = ctx.enter_context(tc.tile_pool(name="ostage", bufs=3))
        ps_a = ctx.enter_context(tc.tile_pool(name="psA", bufs=2, space="PSUM"))
        ps_s = ctx.enter_context(tc.tile_pool(name="psS", bufs=2, space="PSUM"))
        ps_v = ctx.enter_context(tc.tile_pool(name="psV", bufs=2, space="PSUM"))
        ps_m = ctx.enter_context(tc.tile_pool(name="psM", bufs=2, space="PSUM"))

        # ---- constants & first loads (issue order = DMA FIFO order) ----
        wts = [wpool.tile([128, NCH * 3 * D], bf16, tag="wh", name=f"wh{h}")
               for h in range(HPC)]

        def load_wh(h, eng):
            eng.dma_start(
                wts[h][:, :].rearrange("p (c f) -> p c f", f=3 * D),
                wqkvT[:, h * 3 * D:(h + 1) * 3 * D].rearrange(
                    "(c p) f -> p c f", p=128))

        # The first QKV psum group accumulates over ALL 16 x-chunks, so the
        # whole x must land before the first matmul group can finish: split
        # the x load across both HWDGE rings, weights/constants behind them.
        xg = [xpool.tile([128, 4 * L], bf16, tag=f"xg{g}", name=f"xg{g}")
              for g in range(4)]

        def load_xg(g, eng):
            eng.dma_start(xg[g][:, :].rearrange("p (c l) -> p c l", l=L),
                          xT[g * 512:(g + 1) * 512].rearrange(
                              "(c p) l -> p c l", p=128))

        load_xg(0, nc.sync)
        load_xg(1, nc.scalar)
        load_wh(0, nc.sync)
        load_xg(2, nc.sync)
        load_xg(3, nc.scalar)
        csb = cpool.tile([128, L], bf16, tag="csb")
        nc.scalar.dma_start(csb[:, :], csb_d[:, :])
        csa = cpool.tile([128, L], bf16, tag="csa")
        nc.scalar.dma_start(csa[:, :], csa_d[:, :])
        load_wh(1, nc.scalar)
        wo = cpool.tile([128, HPC * HID], bf16, tag="wo")
        nc.scalar.dma_start(wo[:, :].rearrange("p (h f) -> p h f", f=HID),
                            woutT.rearrange("(h p) f -> p h f", p=128))

        ident = cpool.tile([128, 128], bf16, tag="ident")
        from concourse.masks import make_identity
        make_identity(nc, ident[:, :])
        # mask128[k, q] = 0 where q >= k else MASK_VAL (strict upper = masked)
        mask128 = cpool.tile([128, 128], f32, tag="mask128")
        nc.gpsimd.memset(mask128[:, :], 0.0)
        nc.gpsimd.affine_select(
            out=mask128[:, :], in_=mask128[:, :],
            compare_op=mybir.AluOpType.is_ge, fill=MASK_VAL,
            base=0, pattern=[[1, 128]], channel_multiplier=-1)
        ones_cb = cpool.tile([128, 1], bf16, tag="ones_cb")
        nc.gpsimd.memset(ones_cb[:, :], 1.0)
        eps_q = cpool.tile([1, 1], f32, tag="eps_q")
        nc.gpsimd.memset(eps_q[:, :], float(D) * EPS)
        eps_k = cpool.tile([128, 1], f32, tag="eps_k")
        nc.gpsimd.memset(eps_k[:, :], EPS)

        houtT = [hpool.tile([128, L], bf16, tag=f"houtT{h}", name=f"houtT{h}")
                 for h in range(HPC)]

        def make_head_tiles(h):
            qr = qkpool.tile([128, L], bf16, tag="qr", name=f"qr{h}")
            kr = qkpool.tile([128, L], bf16, tag="kr", name=f"kr{h}")
            vnat = vpool.tile([128, L], bf16, tag="vnat", name=f"vnat{h}")
            ckT = vpool.tile([128, NCH], f32, tag="ckT", name=f"ckT{h}")
            return qr, kr, vnat, ckT

        def emit_qkv_piece(h, t, name, n, tiles):
            """One (q|k|v, L-block) projection piece: 16 accumulating matmuls
            + RMS/RoPE (q/k) or PE transposes into natural layout (v)."""
            qr, kr, vnat, ckT = tiles
            wcol = t * D
            ps = ps_a.tile([128, NL], f32, tag="a", name=f"ps{h}{t}{n}")
            for c in range(NCH):
                nc.tensor.matmul(
                    ps[:, :],
                    wts[h][:, c * 3 * D + wcol: c * 3 * D + wcol + D],
                    xg[c // 4][:, (c % 4) * L + n * NL:
                               (c % 4) * L + (n + 1) * NL],
                    start=(c == 0), stop=(c == NCH - 1))
            if name == "v":
                # V^T block -> natural-layout V chunks via PE transpose;
                # vnat[:, c*128:...] = [k, d] chunk c.
                vts = spool.tile([128, NL], bf16, tag="vts")
                nc.scalar.copy(vts[:, :], ps[:, :])
                for i in range(4):
                    vtp = ps_m.tile([128, 128], bf16, tag="m",
                                    name=f"vtp{h}_{n}_{i}")
                    nc.tensor.transpose(
                        vtp[:, :], vts[:, i * 128:(i + 1) * 128],
                        ident[:, :])
                    nc.vector.tensor_copy(
                        vnat[:, (4 * n + i) * 128:(4 * n + i + 1) * 128],
                        vtp[:, :])
                return
            # RoPE all-bf16 on DVE (2x/4x DVE modes need 2-byte packed SBUF
            # operands): one PSUM->bf16 copy on ACT, everything downstream
            # bf16.
            xb = spool.tile([128, NL], bf16, tag="xb")
            nc.scalar.copy(xb[:, :], ps[:, :])
            sw = spool.tile([128, NL], bf16, tag="sw")
            nc.vector.tensor_copy(sw[0:64, :], xb[64:128, :])
            nc.vector.tensor_copy(sw[64:128, :], xb[0:64, :])
            sq = spool.tile([128, NL], bf16, tag="sq")
            nc.vector.tensor_mul(sq[:, :], xb[:, :], xb[:, :])
            m2 = spool.tile([128, NL], bf16, tag="m2")
            nc.vector.tensor_mul(m2[:, :], xb[:, :],
                                 csb[:, n * NL:(n + 1) * NL])
            if name == "q":
                # c_q = 1/sqrt(sumsq + D*eps)  (includes 1/sqrt(D))
                rrow = ps_m.tile([1, NL], f32, tag="m", name=f"rrow{h}{n}")
                nc.tensor.matmul(rrow[:, :], ones_cb[:, :], sq[:, :],
                                 start=True, stop=True)
                srow = spool.tile([1, NL], f32, tag="srow", bufs=1)
                nc.scalar.activation(srow[:, :], rrow[:, :], SQRT,
                                     bias=eps_q[:, :], scale=1.0)
                cqrow = spool.tile([1, NL], f32, tag="cqrow", bufs=1)
                nc.vector.reciprocal(cqrow[:, :], srow[:, :])
                bcs = spool.tile([128, NL], f32, tag="bcs", bufs=1)
                nc.gpsimd.partition_broadcast(bcs[:, :], cqrow[:, :])
            else:
                # c_k = 1/sqrt(sumsq/D + eps), in [128, 4] per chunk
                ckp = ps_m.tile([128, 4], f32, tag="m", name=f"ckp{h}{n}")
                for i in range(4):
                    nc.tensor.matmul(
                        ckp[:, i:i + 1],
                        sq[:, i * 128:(i + 1) * 128],
                        ones_cb[:, :],
                        start=True, stop=True, skip_group_check=True)
                cks = spool.tile([128, 4], f32, tag="cks")
                nc.scalar.activation(cks[:, :], ckp[:, :], SQRT,
                                     bias=eps_k[:, :], scale=1.0 / D)
                nc.vector.reciprocal(ckT[:, n * 4:(n + 1) * 4],
                                     cks[:, :])
            # RoPE: y = csa*halfswap(x) + csb*x  [+ *c_q for q]
            m1 = spool.tile([128, NL], bf16, tag="m1")
            nc.vector.tensor_mul(m1[:, :], sw[:, :],
                                 csa[:, n * NL:(n + 1) * NL])
            dst = qr if name == "q" else kr
            if name == "q":
                nc.vector.tensor_add(m2[:, :], m1[:, :], m2[:, :])
                nc.vector.tensor_mul(dst[:, n * NL:(n + 1) * NL],
                                     m2[:, :], bcs[:, :])
            else:
                nc.vector.tensor_add(dst[:, n * NL:(n + 1) * NL],
                                     m1[:, :], m2[:, :])

        QKV_ORDER = [(t, name, n) for t, name in ((0, "q"), (1, "k"), (2, "v"))
                     for n in range(NLB)]

        def emit_scores(h, J, tiles):
            qr, kr, vnat, ckT = tiles
            etiles = []
            for c in range(4 * J + 4):
                r = c - 4 * J
                et = epool.tile([128, NL], bf16, tag="e", name=f"e{h}{J}_{c}")
                if r >= 0:
                    # diagonal-straddling tile: columns below q = 128r fully
                    # masked -- skip in matmul, mask the diagonal block,
                    # zero-fill the dead prefix of E.
                    w = NL - r * 128
                    sp = ps_s.tile([128, NL], f32, tag="s",
                                   name=f"spd{h}{J}_{c}")
                    nc.tensor.matmul(
                        sp[:, 0:w], kr[:, c * 128:(c + 1) * 128],
                        qr[:, J * NL + r * 128:(J + 1) * NL],
                        start=True, stop=True)
                    nc.vector.tensor_add(sp[:, 0:128], sp[:, 0:128],
                                         mask128[:, :])
                    if r > 0:
                        nc.gpsimd.memset(et[:, 0:r * 128], 0.0)
                    nc.scalar.activation(et[:, r * 128:], sp[:, 0:w],
                                         EXP, scale=ckT[:, c:c + 1])
                else:
                    sp = ps_s.tile([128, NL], f32, tag="s",
                                   name=f"sp{h}{J}_{c}")
                    nc.tensor.matmul(sp[:, :], kr[:, c * 128:(c + 1) * 128],
                                     qr[:, J * NL:(J + 1) * NL],
                                     start=True, stop=True)
                    nc.scalar.activation(et[:, :], sp[:, :],
                                         EXP, scale=ckT[:, c:c + 1])
                etiles.append(et)
            return etiles

        def emit_pv_den(h, J, tiles, etiles):
            qr, kr, vnat, ckT = tiles
            # PV first (its early chunks' E tiles are ready before the last
            # exp lands), then the denominator matmuls.
            nch_j = 4 * J + 4
            op = ps_v.tile([128, NL], f32, tag="v", name=f"op{h}{J}")
            for c in range(nch_j):
                nc.tensor.matmul(op[:, :],
                                 vnat[:, c * 128:(c + 1) * 128],
                                 etiles[c][:, :],
                                 start=(c == 0), stop=(c == nch_j - 1))
            # denominator: dp[1, q] = sum_k E^T[k, q]
            dp = ps_m.tile([1, NL], f32, tag="m", name=f"dp{h}{J}")
            for c in range(nch_j):
                nc.tensor.matmul(dp[:, :], ones_cb[:, :],
                                 etiles[c][:, :],
                                 start=(c == 0), stop=(c == nch_j - 1))
            rd = spool.tile([1, NL], f32, tag="rd", bufs=1)
            nc.vector.reciprocal(rd[:, :], dp[:, :])
            dbc = spool.tile([128, NL], f32, tag="dbc", bufs=1)
            nc.gpsimd.partition_broadcast(dbc[:, :], rd[:, :])
            nc.vector.tensor_mul(houtT[h][:, J * NL:(J + 1) * NL],
                                 op[:, :], dbc[:, :])

        # Software pipeline: head h's attention J-blocks interleave with head
        # h+1's projection pieces (emitted between scores and PV so the PE
        # fills the exp-latency bubble with projection matmuls).
        cur = make_head_tiles(0)
        for t, name, n in QKV_ORDER:
            emit_qkv_piece(0, t, name, n, cur)
        for h in range(HPC):
            pending = []
            if h + 1 < HPC:
                if h + 2 < HPC:
                    load_wh(h + 2, nc.scalar)
                nxt = make_head_tiles(h + 1)
                pending = [(t, name, n) for t, name, n in QKV_ORDER]
            for J in range(NLB):
                etiles = emit_scores(h, J, cur)
                take = 3 if J < 3 else len(pending)
                for t, name, n in pending[:take]:
                    emit_qkv_piece(h + 1, t, name, n, nxt)
                pending = pending[take:]
                emit_pv_den(h, J, cur, etiles)
            assert not pending
            if h + 1 < HPC:
                cur = nxt

        # ---- output projection (partial over this core's 512 channels) ----
        # attention psum pools are free here: rotate across all three so the
        # staging copies never block the next matmul group.
        op_pools = [(ps_a, "a"), (ps_s, "s"), (ps_v, "v")]
        for qb in range(L // 128):
            ob = opool.tile([128, L], bf16, tag="ob", name=f"ob{qb}")
            for f in range(NLB):
                pool, ptag = op_pools[(qb * NLB + f) % 3]
                opj = pool.tile([128, NL], f32, tag=ptag, name=f"opj{qb}_{f}")
                for hh in range(HPC):
                    nc.tensor.matmul(
                        opj[:, :],
                        houtT[hh][:, qb * 128:(qb + 1) * 128],
                        wo[:, hh * HID + f * NL: hh * HID + (f + 1) * NL],
                        start=(hh == 0), stop=(hh == HPC - 1))
                eng = nc.scalar if f % 2 == 0 else nc.vector
                if f % 2 == 0:
                    eng.copy(ob[:, f * NL:(f + 1) * NL], opj[:, :])
                else:
                    eng.tensor_copy(ob[:, f * NL:(f + 1) * NL], opj[:, :])
            eng = nc.sync if qb % 2 == 0 else nc.scalar
            eng.dma_start(out_d[qb * 128:(qb + 1) * 128, :], ob[:, :])

    nc.compile()
    _cache["nc"] = nc
    return nc


def _prep_in_maps(x, W_qkv, W_out):
    csa, csb = _host_constants()
    perm = np.concatenate([np.arange(0, D, 2), np.arange(1, D, 2)])
    xTs = [np.ascontiguousarray(np.asarray(x[b]).T).astype(_BF16)
           for b in range(B)]
    in_maps = []
    for core in range(NCORES):
        b, g = core // NG, core % NG
        h0 = HPC * g
        blocks = []
        for h in range(h0, h0 + HPC):
            wq = W_qkv[h * D:(h + 1) * D, :][perm]
            wk = W_qkv[HID + h * D: HID + (h + 1) * D, :][perm]
            wv = W_qkv[2 * HID + h * D: 2 * HID + (h + 1) * D, :]
            blocks += [wq, wk, wv]
        wqkvT = np.ascontiguousarray(
            np.concatenate(blocks, axis=0).T).astype(_BF16)
        woutT = np.ascontiguousarray(
            W_out[:, h0 * D:(h0 + HPC) * D].T).astype(_BF16)
        in_maps.append({
            "xT": xTs[b], "wqkvT": wqkvT, "woutT": woutT,
            "csa": csa, "csb": csb,
        })
    return in_maps


def kernel(x, W_qkv, W_out):
    from concourse.bass_utils import run_bass_kernel_spmd
    nc = _build_nc()
    in_maps = _prep_in_maps(np.asarray(x, dtype=np.float32),
                            np.asarray(W_qkv, dtype=np.float32),
                            np.asarray(W_out, dtype=np.float32))
    res = run_bass_kernel_spmd(nc, in_maps, core_ids=list(range(NCORES)))
    out = np.empty((B, L, HID), np.float32)
    for b in range(B):
        acc = res.results[b * NG]["outp"].astype(np.float32)
        for g in range(1, NG):
            acc = acc + res.results[b * NG + g]["outp"].astype(np.float32)
        out[b] = acc
    return out
